# revision 35
# baseline (speedup 1.0000x reference)
"""GQA attention block (B=2, N=2048, D=2048, Hq=32, Hkv=8, d=64) on 8 TRN2 NeuronCores.

Sharding: core c = b*4 + hg  (data-parallel over batch b in {0,1}; tensor-parallel
over 4 head-groups hg, each owning 8 q-heads / 2 kv-heads).  Each core computes a
row-parallel partial of the output projection for its batch; the host sums the 4
partials per batch.

Per-core device pipeline (matmuls in bf16, PSUM accumulation fp32):
  1) token-major fused QKV projection:  psum[tok128, 768] = x_chunk.T @ Wqkv.T
  2) RMSNorm (free-dim segment reduce) + NeoX RoPE via 4 host-folded tables
     (q tables also absorb the 1/sqrt(d) score scale and q_norm_w; k tables absorb
     k_norm_w) in fp32
  3) PE transpose of rotated q,k to feature-major, cast to bf16 on the copy out
  4) per (head, 512-token q-chunk): scoresT[k,q] matmuls (K=64) in bf16, exp on
     ScalarE (no max-subtraction: scores are O(5) so exp is safe) -> bf16, causal
     mask via GPSIMD affine_select, PV matmul with an appended ones-column on V
     producing y and the softmax denominator in one accumulation; normalize with
     DVE reciprocal + a K=1 bf16 broadcast matmul
  5) out-projection partial (bf16 matmul), token-major, DMA'd out in fp32
"""

import numpy as np

D_MODEL = 2048
H_Q, H_KV, D_HEAD = 32, 8, 64
B = 2
N = 2048
ROPE_BASE = 10000.0
EPS = 1e-6
NCORES = 8
P = 128


def _modules():
    import sys

    for p in ("/opt/trn_rl_repo",):
        if p not in sys.path:
            sys.path.insert(0, p)
    import concourse.bass as bass
    import concourse.tile as tile
    from concourse import bacc, mybir
    from concourse.masks import make_identity

    return bass, tile, bacc, mybir, make_identity


def build_nc(n_tok=N, causal=True, dbg=False):
    """Build the single-core SPMD Bass program (identical on all 8 cores)."""
    from contextlib import ExitStack

    bass, tile, bacc, mybir, make_identity = _modules()
    f32 = mybir.dt.float32
    bf16 = mybir.dt.bfloat16
    ts = bass.ts
    AF = mybir.ActivationFunctionType
    OP = mybir.AluOpType

    NT = n_tok // P           # token tiles
    DC = D_MODEL // P         # contraction chunks for qkv proj
    QC = n_tok // 512         # query chunks of 512
    NG = n_tok // 512         # x-load groups (512 tokens each)
    assert QC >= 1 and n_tok % 512 == 0

    nc = bacc.Bacc("TRN2", target_bir_lowering=False, debug=False,
                   num_devices=NCORES)

    xT = nc.dram_tensor("xT", [D_MODEL, n_tok], bf16, kind="ExternalInput").ap()
    wqkv = nc.dram_tensor("wqkv", [D_MODEL, 768], bf16, kind="ExternalInput").ap()
    wo = nc.dram_tensor("wo", [512, D_MODEL], bf16, kind="ExternalInput").ap()
    tabq = nc.dram_tensor("tabq", [P, NT, 4, 32], f32, kind="ExternalInput").ap()
    tabk = nc.dram_tensor("tabk", [P, NT, 4, 32], f32, kind="ExternalInput").ap()
    out = nc.dram_tensor("out", [n_tok, D_MODEL], bf16,
                         kind="ExternalOutput").ap()

    with ExitStack() as ctx:
        tc = ctx.enter_context(tile.TileContext(nc))

        cpool = ctx.enter_context(tc.tile_pool(name="const", bufs=1))
        # persistent activations (bf16 matmul operands)
        qfm = [cpool.tile([P, n_tok], bf16, name=f"qfm{c}") for c in range(4)]
        kfm = cpool.tile([P, n_tok], bf16, name="kfm")     # [kv0 | kv1] on partitions
        kswap = cpool.tile([P, n_tok], bf16, name="kswap")  # [kv1 | kv0]
        yfm = [cpool.tile([P, n_tok], bf16, name=f"yfm{c}") for c in range(4)]
        vsb = [cpool.tile([P, 130], bf16, name=f"vsb{t}") for t in range(NT)]
        ident = cpool.tile([P, P], f32, name="ident")
        make_identity(nc, ident[:])
        ones_bf = cpool.tile([1, 64], bf16, name="ones_bf")
        nc.gpsimd.memset(ones_bf[:], 1.0)
        eps_t = cpool.tile([P, 1], f32, name="eps_t")
        nc.gpsimd.memset(eps_t[:], EPS)
        for t in range(NT):
            nc.gpsimd.memset(vsb[t][:, 64:65], 1.0)
            nc.gpsimd.memset(vsb[t][:, 129:130], 1.0)

        # ---------------- phase 1: qkv + norm + rope + transpose ----------
        with ExitStack() as p1:
            wpool = p1.enter_context(tc.tile_pool(name="wqkv", bufs=1))
            tpool = p1.enter_context(tc.tile_pool(name="tabs", bufs=1))
            xpool = p1.enter_context(tc.tile_pool(name="xg", bufs=17))
            wkk = p1.enter_context(tc.tile_pool(name="qkvwork", bufs=2))
            qkv_ps = p1.enter_context(
                tc.tile_pool(name="qkvpsum", bufs=2, space="PSUM"))
            tp_ps = p1.enter_context(
                tc.tile_pool(name="tppsum", bufs=2, space="PSUM"))

            # DMA order matters: the first QKV matmul needs x group 0 and the
            # first weight chunk, so issue those ahead of the big table loads
            # (all DMAs drain through one issue queue).  Weights are one tile
            # per contraction chunk so the accumulation can start as soon as
            # chunk 0 lands (tile-granular dependency tracking).
            wq_sb = [wpool.tile([P, 768], bf16, name=f"wq{dc}")
                     for dc in range(DC)]
            wq_src = wqkv.rearrange("(o p) r -> p o r", p=P)
            xg0 = [xpool.tile([P, 256], bf16, tag="xg", name=f"xg0_{dc}")
                   for dc in range(DC)]
            for dc in range(DC):
                nc.sync.dma_start(xg0[dc][:], xT[ts(dc, P), ts(0, 256)])
                nc.sync.dma_start(wq_sb[dc][:], wq_src[:, dc, :])
            # x group 1 queued right behind group 0 + weights so the PE never
            # waits on it; rope tables (4 chunk-tiles each) follow — tile 0's
            # rope only gates on the first chunk
            xg1 = [xpool.tile([P, 256], bf16, tag="xg", name=f"xg1_{dc}")
                   for dc in range(DC)]
            for dc in range(DC):
                nc.sync.dma_start(xg1[dc][:], xT[ts(dc, P), ts(1, 256)])
            TC4 = NT // 4
            tq = [tpool.tile([P, TC4, 4, 32], f32, name=f"tq{i}")
                  for i in range(4)]
            tk = [tpool.tile([P, TC4, 4, 32], f32, name=f"tk{i}")
                  for i in range(4)]
            nc.sync.dma_start(tq[0][:], tabq[:, ts(0, TC4)])
            nc.sync.dma_start(tk[0][:], tabk[:, ts(0, TC4)])

            for g in range(2 * NG):
                if g == 0:
                    xg = xg0
                elif g == 1:
                    xg = xg1
                else:
                    xg = [xpool.tile([P, 256], bf16, tag="xg",
                                     name=f"xg{g}_{dc}")
                          for dc in range(DC)]
                    for dc in range(DC):
                        nc.sync.dma_start(xg[dc][:], xT[ts(dc, P), ts(g, 256)])
                if g == 1:
                    for q4 in range(1, 4):
                        nc.sync.dma_start(tq[q4][:], tabq[:, ts(q4, TC4)])
                        nc.sync.dma_start(tk[q4][:], tabk[:, ts(q4, TC4)])
                for lt in range(2):
                    tt = g * 2 + lt
                    ps = qkv_ps.tile([P, 768], f32, tag="qkv")
                    for dc in range(DC):
                        lhsT = xg[dc][:, ts(lt, P)]
                        nc.tensor.matmul(ps[:, 0:512], lhsT,
                                         wq_sb[dc][:, 0:512],
                                         start=(dc == 0), stop=(dc == DC - 1))
                        nc.tensor.matmul(ps[:, 512:768], lhsT,
                                         wq_sb[dc][:, 512:768],
                                         start=(dc == 0), stop=(dc == DC - 1))
                    # --- rmsnorm ---
                    sq = wkk.tile([P, 512], f32, tag="sq")
                    nc.scalar.activation(sq[:], ps[:, 0:512], AF.Square)
                    sqk = wkk.tile([P, 128], f32, tag="sqk")
                    nc.scalar.activation(sqk[:], ps[:, 512:640], AF.Square)
                    ssq = wkk.tile([P, 10], f32, tag="ssq")
                    nc.vector.reduce_sum(
                        ssq[:, 0:8], sq[:].rearrange("p (h d) -> p h d", d=64),
                        axis=mybir.AxisListType.X)
                    nc.vector.reduce_sum(
                        ssq[:, 8:10], sqk[:].rearrange("p (h d) -> p h d", d=64),
                        axis=mybir.AxisListType.X)
                    sd = wkk.tile([P, 10], f32, tag="sd")
                    nc.scalar.activation(sd[:], ssq[:], AF.Sqrt,
                                         bias=eps_t[:], scale=1.0 / 64)
                    rs = wkk.tile([P, 10], f32, tag="rs")
                    nc.vector.reciprocal_approx_fast(rs[:], sd[:])
                    qn = wkk.tile([P, 512], f32, tag="sq")
                    nc.vector.tensor_tensor(
                        qn[:].rearrange("p (h d) -> p h d", d=64),
                        ps[:, 0:512].rearrange("p (h d) -> p h d", d=64),
                        rs[:, 0:8, None].to_broadcast([P, 8, 64]), OP.mult)
                    kn = wkk.tile([P, 128], f32, tag="kn")
                    nc.vector.tensor_tensor(
                        kn[:].rearrange("p (h d) -> p h d", d=64),
                        ps[:, 512:640].rearrange("p (h d) -> p h d", d=64),
                        rs[:, 8:10, None].to_broadcast([P, 2, 64]), OP.mult)
                    # --- v copy (bf16 cast, with ones cols at 64 and 129) ---
                    nc.scalar.activation(vsb[tt][:, 0:64], ps[:, 640:704], AF.Copy)
                    nc.scalar.activation(vsb[tt][:, 65:129], ps[:, 704:768], AF.Copy)
                    # --- rope (fp32) ---
                    qr = wkk.tile([P, 512], f32, tag="qr")
                    kr = wkk.tile([P, 128], f32, tag="kr")
                    for (src, dst, tabs, nh) in ((qn, qr, tq, 8),
                                                 (kn, kr, tk, 2)):
                        sv = src[:].rearrange("p (h d) -> p h d", d=64)
                        dv = dst[:].rearrange("p (h d) -> p h d", d=64)
                        t1, t2 = sv[:, :, 0:32], sv[:, :, 32:64]
                        tab, tl = tabs[tt // TC4], tt % TC4
                        A = tab[:, tl, 0:1, :].to_broadcast([P, nh, 32])
                        Bt = tab[:, tl, 1:2, :].to_broadcast([P, nh, 32])
                        C = tab[:, tl, 2:3, :].to_broadcast([P, nh, 32])
                        D = tab[:, tl, 3:4, :].to_broadcast([P, nh, 32])
                        u1 = wkk.tile([P, nh, 32], f32, tag=f"u1_{nh}")
                        u2 = wkk.tile([P, nh, 32], f32, tag=f"u2_{nh}")
                        nc.vector.tensor_tensor(u1[:], t1, A, OP.mult)
                        nc.vector.tensor_tensor(u2[:], t2, Bt, OP.mult)
                        nc.vector.tensor_tensor(dv[:, :, 0:32], u1[:], u2[:],
                                                OP.subtract)
                        u3 = wkk.tile([P, nh, 32], f32, tag=f"u1_{nh}")
                        u4 = wkk.tile([P, nh, 32], f32, tag=f"u2_{nh}")
                        nc.vector.tensor_tensor(u3[:], t1, C, OP.mult)
                        nc.vector.tensor_tensor(u4[:], t2, D, OP.mult)
                        nc.vector.tensor_tensor(dv[:, :, 32:64], u3[:], u4[:],
                                                OP.add)
                    # --- transpose to feature-major (fp32 PE transpose,
                    #     bf16 cast on the PSUM->SBUF copy) ---
                    for rc in range(4):
                        pt = tp_ps.tile([P, P], f32, tag="tp")
                        nc.tensor.transpose(pt[:], qr[:, ts(rc, P)], ident[:])
                        nc.vector.tensor_copy(qfm[rc][:, ts(tt, P)], pt[:])
                    pt = tp_ps.tile([P, P], f32, tag="tp")
                    nc.tensor.transpose(pt[:], kr[:], ident[:])
                    nc.vector.tensor_copy(kfm[:, ts(tt, P)], pt[:])
            # kswap = partition halves of kfm exchanged (SBUF->SBUF DMA)
            nc.sync.dma_start(kswap[64:128, :], kfm[0:64, :])
            nc.sync.dma_start(kswap[0:64, :], kfm[64:128, :])

        # ---------------- phase 2: attention --------------------------
        wopool = ctx.enter_context(tc.tile_pool(name="wo", bufs=1))
        # allocated here (after phase-1 pools closed) so its SBUF reservation
        # does not overlap the phase-1 peak
        wo_sb = wopool.tile([P, 4, D_MODEL], bf16, name="wo_sb")
        nc.sync.dma_start(wo_sb[:], wo.rearrange("(o p) d -> p o d", p=P))
        with ExitStack() as p2:
            epool = p2.enter_context(tc.tile_pool(name="exp", bufs=4))
            npool = p2.enter_context(tc.tile_pool(name="nrm", bufs=4))
            opool = p2.enter_context(tc.tile_pool(name="osb", bufs=3))
            s_ps = p2.enter_context(
                tc.tile_pool(name="spsum", bufs=2, space="PSUM"))
            y_ps = p2.enter_context(
                tc.tile_pool(name="ypsum", bufs=2, space="PSUM"))
            r_ps = p2.enter_context(
                tc.tile_pool(name="rpsum", bufs=1, space="PSUM"))
            o_ps = p2.enter_context(
                tc.tile_pool(name="opsum", bufs=1, space="PSUM"))

            def issue_scores(ksrc, c, p0, qc, pp):
                ps_s = s_ps.tile([P, 1024], f32, tag="s")
                for j in range(2):
                    kt = pp * 2 + j
                    nc.tensor.matmul(
                        ps_s[:, ts(j, 512)],
                        ksrc[p0:p0 + 64, ts(kt, P)],
                        qfm[c][p0:p0 + 64, ts(qc, 512)],
                        start=True, stop=True)
                return ps_s

            # Deferred softmax-normalize of the previous (h, qc) group: 1/den
            # on DVE, broadcast to 64 partitions via a K=1 bf16 matmul.  (DVE
            # ops may read at most ONE PSUM operand, so y is bounced through
            # SBUF.)  Called from inside the NEXT group's pair loop so the
            # broadcast matmul does not stall the in-order PE queue.
            pending = []

            def flush_normalize():
                while pending:
                    ps_y, c, p0, qc = pending.pop()
                    draw = npool.tile([65, 512], f32, tag="draw")
                    nc.vector.tensor_copy(draw[:], ps_y[:])
                    # reciprocal_approx_* misbehaves off partition 0, so hop
                    # the denominator row down first (cross-partition copy)
                    den0 = npool.tile([1, 512], f32, tag="den0")
                    nc.vector.tensor_copy(den0[:], draw[64:65, :])
                    rcp = npool.tile([1, 512], f32, tag="rcp")
                    nc.vector.reciprocal_approx_fast(rcp[:], den0[:])
                    den_bf = npool.tile([1, 512], bf16, tag="denb")
                    nc.vector.tensor_copy(den_bf[:], rcp[:])
                    ps_r = r_ps.tile([64, 512], f32, tag="r")
                    nc.tensor.matmul(ps_r[:], ones_bf[:], den_bf[:],
                                     start=True, stop=True)
                    nc.vector.tensor_tensor(yfm[c][p0:p0 + 64, ts(qc, 512)],
                                            draw[0:64, :], ps_r[:], OP.mult)

            # Out-projection, fused into the attention stream.  A group is
            # one (token tile, 512-col chunk): 4 accumulating matmuls + a DVE
            # PSUM->SBUF bounce + the output DMA.  Groups for q-chunk qc
            # become ready once all 8 heads have normalized qc; they are
            # interleaved one per pair-iteration of the NEXT qc as PE filler
            # (keeps the HAM clock gate warm through exp stalls and absorbs
            # what used to be a serial phase 3).
            owork = []

            def emit_outproj_group():
                t, oc = owork.pop(0)
                ps_o = o_ps.tile([P, 512], f32, tag="o")
                for yc in range(4):
                    nc.tensor.matmul(ps_o[:], yfm[yc][:, ts(t, P)],
                                     wo_sb[:, yc, ts(oc, 512)],
                                     start=(yc == 0), stop=(yc == 3))
                ob = opool.tile([P, 512], bf16, tag="ob")
                nc.vector.tensor_copy(ob[:], ps_o[:])
                nc.sync.dma_start(out[ts(t, P), ts(oc, 512)], ob[:])

            # flat (group, pair) schedule: scores for pair s+1 are issued
            # between exp(s) and PV(s) — across group boundaries too — so the
            # PE always has runway while ScalarE computes the current exp
            groups = []
            for qc in range(QC):
                # kswap is ready only a little after phase 1; run the heads
                # that read kfm directly first on the opening q-chunk
                horder = (0, 2, 5, 7, 1, 3, 4, 6) if qc == 0 else range(8)
                for h in horder:
                    kv, c, p0 = h // 4, h // 2, 64 * (h % 2)
                    nat = (kv == 0) == (p0 == 0)
                    nkt = 4 * qc + 4 if causal else 4 * QC
                    groups.append((qc, kv, c, p0,
                                   kfm if nat else kswap, nkt))
            sched = [(gi, pp) for gi, g in enumerate(groups)
                     for pp in range(g[5] // 2)]

            def issue_scores2(gi, pp):
                qc, kv, c, p0, ksrc, nkt = groups[gi]
                return issue_scores(ksrc, c, p0, qc, pp)

            pace = [0, 1]  # Bresenham accumulator / slots-per-emit denom
            ps_y_of = {}
            slot_s = {0: issue_scores2(*sched[0])}
            for si, (gi, pp) in enumerate(sched):
                qc, kv, c, p0, ksrc, nkt = groups[gi]
                npairs = nkt // 2
                ps_s = slot_s.pop(si)
                eg = epool.tile([P, 1024], bf16, tag="eg")
                nc.scalar.activation(eg[:], ps_s[:], AF.Exp)
                if causal and pp >= npairs - 2:
                    ppl = pp - (npairs - 2)  # 0 or 1 within diag quad
                    # keep where ktok <= q  <=>  q - i - 128m >= 0 for diag
                    # k-tile m; only columns < 128(m+1) can be masked, so
                    # select on just that prefix (cheaper + lower latency)
                    for j in range(2):
                        m = 2 * ppl + j
                        W = 128 * (m + 1)
                        nc.gpsimd.affine_select(
                            eg[:, 512 * j:512 * j + W],
                            eg[:, 512 * j:512 * j + W],
                            pattern=[[1, W]],
                            compare_op=OP.is_ge,
                            fill=0.0,
                            base=-128 * m,
                            channel_multiplier=-1)
                if si + 1 < len(sched):
                    slot_s[si + 1] = issue_scores2(*sched[si + 1])
                if pp == 1:
                    flush_normalize()
                elif owork and not pending:
                    # out-proj filler; valid only once the owning q-chunk's
                    # last normalize has flushed (pending empty)
                    pace[0] += 16
                    if pace[0] >= pace[1]:
                        pace[0] -= pace[1]
                        emit_outproj_group()
                if gi not in ps_y_of:
                    ps_y_of[gi] = y_ps.tile([65, 512], f32, tag="y",
                                            name=f"psy{gi}")
                ps_y = ps_y_of[gi]
                for j in range(2):
                    kt = pp * 2 + j
                    nc.tensor.matmul(
                        ps_y[:], vsb[kt][:, 65 * kv:65 * kv + 65],
                        eg[:, ts(j, 512)],
                        start=(kt == 0), stop=(kt == nkt - 1))
                if pp == npairs - 1:
                    pending.append((ps_y_of.pop(gi), c, p0, qc))
                    if gi + 1 == len(groups) or groups[gi + 1][0] != qc:
                        # qc complete: queue its out-projection groups and
                        # retune pacing for the next q-chunk's slot count
                        owork.extend((4 * qc + t4, oc)
                                     for t4 in range(4)
                                     for oc in range(D_MODEL // 512))
                        pace = [0, max(1, 8 * (2 * (qc + 1) + 1) - 1)]
            flush_normalize()
        # drain the last q-chunk's out-projection with deeper PSUM buffering
        # (the attention PSUM pools are closed, freeing their banks)
        with ExitStack() as p3:
            dpool = p3.enter_context(tc.tile_pool(name="drain", bufs=4))
            d_ps = p3.enter_context(
                tc.tile_pool(name="dpsum", bufs=4, space="PSUM"))
            while owork:
                t, oc = owork.pop(0)
                ps_o = d_ps.tile([P, 512], f32, tag="o")
                for yc in range(4):
                    nc.tensor.matmul(ps_o[:], yfm[yc][:, ts(t, P)],
                                     wo_sb[:, yc, ts(oc, 512)],
                                     start=(yc == 0), stop=(yc == 3))
                ob = dpool.tile([P, 512], bf16, tag="ob")
                nc.vector.tensor_copy(ob[:], ps_o[:])
                nc.sync.dma_start(out[ts(t, P), ts(oc, 512)], ob[:])

    nc.compile()
    return nc


def _rope_tables(pos, norm_w, scale):
    """Build [P, NT, 4, 32] tables A,B,C,D for out1 = t1*A - t2*B,
    out2 = t1*C + t2*D (NeoX rope with folded norm weight + score scale)."""
    n_tok = pos.shape[0]
    f = np.arange(0, D_HEAD, 2, dtype=np.float64) / D_HEAD
    inv_freq = 1.0 / (ROPE_BASE ** f)                       # [32]
    ang = pos.astype(np.float64)[:, None] * inv_freq[None, :]  # [n, 32]
    cos, sin = np.cos(ang), np.sin(ang)
    w1 = norm_w[:32].astype(np.float64)
    w2 = norm_w[32:].astype(np.float64)
    A = cos * w1 * scale
    Bt = sin * w2 * scale
    C = sin * w1 * scale
    D = cos * w2 * scale
    tab = np.stack([A, Bt, C, D], axis=1).astype(np.float32)  # [n, 4, 32]
    return np.ascontiguousarray(
        tab.reshape(n_tok // P, P, 4, 32).transpose(1, 0, 2, 3))


def make_in_maps(x, pos, qkv_w, out_w, q_norm_w, k_norm_w, n_tok=N):
    import ml_dtypes
    bf = ml_dtypes.bfloat16

    scale = D_HEAD ** -0.5
    tabq = _rope_tables(pos, q_norm_w, scale)
    tabk = _rope_tables(pos, k_norm_w, 1.0)
    wq_all = qkv_w[0:H_Q * D_HEAD].reshape(H_Q, D_HEAD, D_MODEL)
    wk_all = qkv_w[H_Q * D_HEAD:(H_Q + H_KV) * D_HEAD].reshape(
        H_KV, D_HEAD, D_MODEL)
    wv_all = qkv_w[(H_Q + H_KV) * D_HEAD:].reshape(H_KV, D_HEAD, D_MODEL)
    wo_all = out_w.reshape(D_MODEL, H_Q, D_HEAD)

    in_maps = []
    for c in range(NCORES):
        b, hg = divmod(c, 4)
        heads = list(range(8 * hg, 8 * hg + 8))
        kvs = [2 * hg, 2 * hg + 1]
        wsel = np.concatenate([
            wq_all[heads].reshape(512, D_MODEL),
            wk_all[kvs].reshape(128, D_MODEL),
            wv_all[kvs].reshape(128, D_MODEL)], axis=0)    # [768, D]
        in_maps.append({
            "xT": np.ascontiguousarray(x[b].T).astype(bf),
            "wqkv": np.ascontiguousarray(wsel.T).astype(bf),
            "wo": np.ascontiguousarray(
                wo_all[:, heads].reshape(D_MODEL, 512).T).astype(bf),
            "tabq": tabq,
            "tabk": tabk,
        })
    return in_maps


def _reference_host(x, mask, pos, qkv_w, out_w, q_norm_w, k_norm_w):
    """Pure-numpy fallback, used only if the mask is not causal."""
    xx = x.astype(np.float64)
    qkv = xx @ qkv_w.T.astype(np.float64)
    Bsz, Nl, _ = x.shape
    qkv = qkv.reshape(Bsz, Nl, H_Q + 2 * H_KV, D_HEAD).transpose(0, 2, 1, 3)
    q, k, v = (qkv[:, :H_Q], qkv[:, H_Q:H_Q + H_KV], qkv[:, H_Q + H_KV:])

    def rms(t, w):
        var = np.mean(t * t, axis=-1, keepdims=True)
        return t / np.sqrt(var + EPS) * w

    def rope(t):
        f = np.arange(0, D_HEAD, 2) / D_HEAD
        inv = 1.0 / (ROPE_BASE ** f)
        ang = pos.astype(np.float64)[:, None] * inv[None, :]
        cs, sn = np.cos(ang), np.sin(ang)
        t1, t2 = t[..., :32], t[..., 32:]
        return np.concatenate([t1 * cs - t2 * sn, t1 * sn + t2 * cs], axis=-1)

    q, k = rope(rms(q, q_norm_w)), rope(rms(k, k_norm_w))
    qg = q.reshape(Bsz, H_KV, 4, Nl, D_HEAD)
    sc = np.einsum("bhgnd,bhmd->bhgnm", qg, k) * (D_HEAD ** -0.5)
    sc = np.where(mask[None, None, None], -np.inf, sc)
    sc -= sc.max(axis=-1, keepdims=True)
    p = np.exp(sc)
    p /= p.sum(axis=-1, keepdims=True)
    y = np.einsum("bhgnm,bhmd->bhgnd", p, v)
    y = y.reshape(Bsz, H_Q, Nl, D_HEAD).transpose(0, 2, 1, 3).reshape(
        Bsz, Nl, D_MODEL)
    return (y @ out_w.T.astype(np.float64)).astype(np.float32)


_NC_CACHE = {}


def run_on_device(in_maps, n_tok=N, trace=False, trace_kwargs=None):
    import sys
    for p in ("/opt/trn_rl_repo",):
        if p not in sys.path:
            sys.path.insert(0, p)
    from concourse.bass_utils import run_bass_kernel_spmd

    key = n_tok
    if key not in _NC_CACHE:
        _NC_CACHE[key] = build_nc(n_tok)
    nc = _NC_CACHE[key]
    return run_bass_kernel_spmd(
        nc, in_maps, list(range(len(in_maps))), trace=trace,
        **(trace_kwargs or {}))


def kernel(x, mask, pos, qkv_w, out_w, q_norm_w, k_norm_w):
    x = np.asarray(x, dtype=np.float32)
    mask = np.asarray(mask)
    pos = np.asarray(pos)
    causal = bool(
        np.array_equal(mask,
                       np.triu(np.ones((N, N), dtype=bool), k=1)))
    if not causal:
        return _reference_host(x, mask, pos, np.asarray(qkv_w),
                               np.asarray(out_w), np.asarray(q_norm_w),
                               np.asarray(k_norm_w))
    in_maps = make_in_maps(x, pos, np.asarray(qkv_w, dtype=np.float32),
                           np.asarray(out_w, dtype=np.float32),
                           np.asarray(q_norm_w, dtype=np.float32),
                           np.asarray(k_norm_w, dtype=np.float32))
    res = run_on_device(in_maps)
    outs = [np.asarray(r["out"], dtype=np.float32) for r in res.results]
    full = np.empty((B, N, D_MODEL), dtype=np.float32)
    for b in range(B):
        full[b] = outs[4 * b] + outs[4 * b + 1] + outs[4 * b + 2] + outs[4 * b + 3]
    return full


# revision 39
# speedup vs baseline: 1.0016x; 1.0016x over previous
"""GQA attention block (B=2, N=2048, D=2048, Hq=32, Hkv=8, d=64) on 8 TRN2 NeuronCores.

Sharding: core c = b*4 + hg  (data-parallel over batch b in {0,1}; tensor-parallel
over 4 head-groups hg, each owning 8 q-heads / 2 kv-heads).  Each core computes a
row-parallel partial of the output projection for its batch; the host sums the 4
partials per batch.

Per-core device pipeline (matmuls in bf16, PSUM accumulation fp32):
  1) token-major fused QKV projection:  psum[tok128, 768] = x_chunk.T @ Wqkv.T
  2) RMSNorm (free-dim segment reduce) + NeoX RoPE via 4 host-folded tables
     (q tables also absorb the 1/sqrt(d) score scale and q_norm_w; k tables absorb
     k_norm_w) in fp32
  3) PE transpose of rotated q,k to feature-major, cast to bf16 on the copy out
  4) per (head, 512-token q-chunk): scoresT[k,q] matmuls (K=64) in bf16, exp on
     ScalarE (no max-subtraction: scores are O(5) so exp is safe) -> bf16, causal
     mask via GPSIMD affine_select, PV matmul with an appended ones-column on V
     producing y and the softmax denominator in one accumulation; normalize with
     DVE reciprocal + a K=1 bf16 broadcast matmul
  5) out-projection partial (bf16 matmul), interleaved into the attention
     stream as PE filler, token-major, DMA'd out in bf16 (host sums in fp32)
"""

import numpy as np

D_MODEL = 2048
H_Q, H_KV, D_HEAD = 32, 8, 64
B = 2
N = 2048
ROPE_BASE = 10000.0
EPS = 1e-6
NCORES = 8
P = 128


def _modules():
    import sys

    for p in ("/opt/trn_rl_repo",):
        if p not in sys.path:
            sys.path.insert(0, p)
    import concourse.bass as bass
    import concourse.tile as tile
    from concourse import bacc, mybir
    from concourse.masks import make_identity

    return bass, tile, bacc, mybir, make_identity


def build_nc(n_tok=N, causal=True, dbg=False):
    """Build the single-core SPMD Bass program (identical on all 8 cores)."""
    from contextlib import ExitStack

    bass, tile, bacc, mybir, make_identity = _modules()
    f32 = mybir.dt.float32
    bf16 = mybir.dt.bfloat16
    ts = bass.ts
    AF = mybir.ActivationFunctionType
    OP = mybir.AluOpType

    NT = n_tok // P           # token tiles
    DC = D_MODEL // P         # contraction chunks for qkv proj
    QC = n_tok // 512         # query chunks of 512
    NG = n_tok // 512         # x-load groups (512 tokens each)
    assert QC >= 1 and n_tok % 512 == 0

    nc = bacc.Bacc("TRN2", target_bir_lowering=False, debug=False,
                   num_devices=NCORES)

    xT = nc.dram_tensor("xT", [D_MODEL, n_tok], bf16, kind="ExternalInput").ap()
    wqkv = nc.dram_tensor("wqkv", [D_MODEL, 768], bf16, kind="ExternalInput").ap()
    wo = nc.dram_tensor("wo", [512, D_MODEL], bf16, kind="ExternalInput").ap()
    tabq = nc.dram_tensor("tabq", [P, NT, 4, 32], f32, kind="ExternalInput").ap()
    tabk = nc.dram_tensor("tabk", [P, NT, 4, 32], f32, kind="ExternalInput").ap()
    out = nc.dram_tensor("out", [n_tok, D_MODEL], bf16,
                         kind="ExternalOutput").ap()

    with ExitStack() as ctx:
        tc = ctx.enter_context(tile.TileContext(nc))

        cpool = ctx.enter_context(tc.tile_pool(name="const", bufs=1))
        # persistent activations (bf16 matmul operands)
        qfm = [cpool.tile([P, n_tok], bf16, name=f"qfm{c}") for c in range(4)]
        kfm = cpool.tile([P, n_tok], bf16, name="kfm")     # [kv0 | kv1] on partitions
        kswap = cpool.tile([P, n_tok], bf16, name="kswap")  # [kv1 | kv0]
        yfm = [cpool.tile([P, n_tok], bf16, name=f"yfm{c}") for c in range(4)]
        vsb = [cpool.tile([P, 130], bf16, name=f"vsb{t}") for t in range(NT)]
        ident = cpool.tile([P, P], f32, name="ident")
        make_identity(nc, ident[:])
        ones_bf = cpool.tile([1, 64], bf16, name="ones_bf")
        nc.gpsimd.memset(ones_bf[:], 1.0)
        eps_t = cpool.tile([P, 1], f32, name="eps_t")
        nc.gpsimd.memset(eps_t[:], EPS)
        for t in range(NT):
            nc.gpsimd.memset(vsb[t][:, 64:65], 1.0)
            nc.gpsimd.memset(vsb[t][:, 129:130], 1.0)

        # ---------------- phase 1: qkv + norm + rope + transpose ----------
        with ExitStack() as p1:
            wpool = p1.enter_context(tc.tile_pool(name="wqkv", bufs=1))
            tpool = p1.enter_context(tc.tile_pool(name="tabs", bufs=1))
            xpool = p1.enter_context(tc.tile_pool(name="xg", bufs=17))
            wkk = p1.enter_context(tc.tile_pool(name="qkvwork", bufs=2))
            qkv_ps = p1.enter_context(
                tc.tile_pool(name="qkvpsum", bufs=2, space="PSUM"))
            tp_ps = p1.enter_context(
                tc.tile_pool(name="tppsum", bufs=2, space="PSUM"))

            # DMA order matters: the first QKV matmul needs x group 0 and the
            # first weight chunk, so issue those ahead of the big table loads
            # (all DMAs drain through one issue queue).  Weights are one tile
            # per contraction chunk so the accumulation can start as soon as
            # chunk 0 lands (tile-granular dependency tracking).
            wq_sb = [wpool.tile([P, 768], bf16, name=f"wq{dc}")
                     for dc in range(DC)]
            wq_src = wqkv.rearrange("(o p) r -> p o r", p=P)
            xg0 = [xpool.tile([P, 256], bf16, tag="xg", name=f"xg0_{dc}")
                   for dc in range(DC)]
            for dc in range(DC):
                nc.sync.dma_start(xg0[dc][:], xT[ts(dc, P), ts(0, 256)])
                nc.sync.dma_start(wq_sb[dc][:], wq_src[:, dc, :])
            # rope tables as 4 chunk-tiles each, so tile 0's rope only gates
            # on the first 512 KiB; later chunks load behind x group 1
            TC4 = NT // 4
            tq = [tpool.tile([P, TC4, 4, 32], f32, name=f"tq{i}")
                  for i in range(4)]
            tk = [tpool.tile([P, TC4, 4, 32], f32, name=f"tk{i}")
                  for i in range(4)]
            nc.sync.dma_start(tq[0][:], tabq[:, ts(0, TC4)])
            nc.sync.dma_start(tk[0][:], tabk[:, ts(0, TC4)])

            for g in range(2 * NG):
                if g == 0:
                    xg = xg0
                else:
                    xg = [xpool.tile([P, 256], bf16, tag="xg",
                                     name=f"xg{g}_{dc}")
                          for dc in range(DC)]
                    for dc in range(DC):
                        nc.sync.dma_start(xg[dc][:], xT[ts(dc, P), ts(g, 256)])
                if g == 1:
                    for q4 in range(1, 4):
                        nc.sync.dma_start(tq[q4][:], tabq[:, ts(q4, TC4)])
                        nc.sync.dma_start(tk[q4][:], tabk[:, ts(q4, TC4)])
                for lt in range(2):
                    tt = g * 2 + lt
                    ps = qkv_ps.tile([P, 768], f32, tag="qkv")
                    for dc in range(DC):
                        lhsT = xg[dc][:, ts(lt, P)]
                        nc.tensor.matmul(ps[:, 0:512], lhsT,
                                         wq_sb[dc][:, 0:512],
                                         start=(dc == 0), stop=(dc == DC - 1))
                        nc.tensor.matmul(ps[:, 512:768], lhsT,
                                         wq_sb[dc][:, 512:768],
                                         start=(dc == 0), stop=(dc == DC - 1))
                    # --- rmsnorm ---
                    sq = wkk.tile([P, 512], f32, tag="sq")
                    nc.scalar.activation(sq[:], ps[:, 0:512], AF.Square)
                    sqk = wkk.tile([P, 128], f32, tag="sqk")
                    nc.scalar.activation(sqk[:], ps[:, 512:640], AF.Square)
                    ssq = wkk.tile([P, 10], f32, tag="ssq")
                    nc.vector.reduce_sum(
                        ssq[:, 0:8], sq[:].rearrange("p (h d) -> p h d", d=64),
                        axis=mybir.AxisListType.X)
                    nc.vector.reduce_sum(
                        ssq[:, 8:10], sqk[:].rearrange("p (h d) -> p h d", d=64),
                        axis=mybir.AxisListType.X)
                    sd = wkk.tile([P, 10], f32, tag="sd")
                    nc.scalar.activation(sd[:], ssq[:], AF.Sqrt,
                                         bias=eps_t[:], scale=1.0 / 64)
                    rs = wkk.tile([P, 10], f32, tag="rs")
                    nc.vector.reciprocal_approx_fast(rs[:], sd[:])
                    qn = wkk.tile([P, 512], f32, tag="sq")
                    nc.vector.tensor_tensor(
                        qn[:].rearrange("p (h d) -> p h d", d=64),
                        ps[:, 0:512].rearrange("p (h d) -> p h d", d=64),
                        rs[:, 0:8, None].to_broadcast([P, 8, 64]), OP.mult)
                    kn = wkk.tile([P, 128], f32, tag="kn")
                    nc.vector.tensor_tensor(
                        kn[:].rearrange("p (h d) -> p h d", d=64),
                        ps[:, 512:640].rearrange("p (h d) -> p h d", d=64),
                        rs[:, 8:10, None].to_broadcast([P, 2, 64]), OP.mult)
                    # --- v copy (bf16 cast, with ones cols at 64 and 129) ---
                    nc.scalar.activation(vsb[tt][:, 0:64], ps[:, 640:704], AF.Copy)
                    nc.scalar.activation(vsb[tt][:, 65:129], ps[:, 704:768], AF.Copy)
                    # --- rope (fp32) ---
                    qr = wkk.tile([P, 512], f32, tag="qr")
                    kr = wkk.tile([P, 128], f32, tag="kr")
                    for (src, dst, tabs, nh) in ((qn, qr, tq, 8),
                                                 (kn, kr, tk, 2)):
                        sv = src[:].rearrange("p (h d) -> p h d", d=64)
                        dv = dst[:].rearrange("p (h d) -> p h d", d=64)
                        t1, t2 = sv[:, :, 0:32], sv[:, :, 32:64]
                        tab, tl = tabs[tt // TC4], tt % TC4
                        A = tab[:, tl, 0:1, :].to_broadcast([P, nh, 32])
                        Bt = tab[:, tl, 1:2, :].to_broadcast([P, nh, 32])
                        C = tab[:, tl, 2:3, :].to_broadcast([P, nh, 32])
                        D = tab[:, tl, 3:4, :].to_broadcast([P, nh, 32])
                        u1 = wkk.tile([P, nh, 32], f32, tag=f"u1_{nh}")
                        u2 = wkk.tile([P, nh, 32], f32, tag=f"u2_{nh}")
                        nc.vector.tensor_tensor(u1[:], t1, A, OP.mult)
                        nc.vector.tensor_tensor(u2[:], t2, Bt, OP.mult)
                        nc.vector.tensor_tensor(dv[:, :, 0:32], u1[:], u2[:],
                                                OP.subtract)
                        u3 = wkk.tile([P, nh, 32], f32, tag=f"u1_{nh}")
                        u4 = wkk.tile([P, nh, 32], f32, tag=f"u2_{nh}")
                        nc.vector.tensor_tensor(u3[:], t1, C, OP.mult)
                        nc.vector.tensor_tensor(u4[:], t2, D, OP.mult)
                        nc.vector.tensor_tensor(dv[:, :, 32:64], u3[:], u4[:],
                                                OP.add)
                    # --- transpose to feature-major (fp32 PE transpose,
                    #     bf16 cast on the PSUM->SBUF copy) ---
                    for rc in range(4):
                        pt = tp_ps.tile([P, P], f32, tag="tp")
                        nc.tensor.transpose(pt[:], qr[:, ts(rc, P)], ident[:])
                        nc.vector.tensor_copy(qfm[rc][:, ts(tt, P)], pt[:])
                    pt = tp_ps.tile([P, P], f32, tag="tp")
                    nc.tensor.transpose(pt[:], kr[:], ident[:])
                    nc.vector.tensor_copy(kfm[:, ts(tt, P)], pt[:])
            # kswap = partition halves of kfm exchanged (SBUF->SBUF DMA)
            nc.sync.dma_start(kswap[64:128, :], kfm[0:64, :])
            nc.sync.dma_start(kswap[0:64, :], kfm[64:128, :])

        # ---------------- phase 2: attention --------------------------
        wopool = ctx.enter_context(tc.tile_pool(name="wo", bufs=1))
        # allocated here (after phase-1 pools closed) so its SBUF reservation
        # does not overlap the phase-1 peak
        wo_sb = wopool.tile([P, 4, D_MODEL], bf16, name="wo_sb")
        nc.sync.dma_start(wo_sb[:], wo.rearrange("(o p) d -> p o d", p=P))
        with ExitStack() as p2:
            epool = p2.enter_context(tc.tile_pool(name="exp", bufs=4))
            npool = p2.enter_context(tc.tile_pool(name="nrm", bufs=4))
            opool = p2.enter_context(tc.tile_pool(name="osb", bufs=3))
            s_ps = p2.enter_context(
                tc.tile_pool(name="spsum", bufs=2, space="PSUM"))
            y_ps = p2.enter_context(
                tc.tile_pool(name="ypsum", bufs=2, space="PSUM"))
            r_ps = p2.enter_context(
                tc.tile_pool(name="rpsum", bufs=1, space="PSUM"))
            o_ps = p2.enter_context(
                tc.tile_pool(name="opsum", bufs=1, space="PSUM"))

            def issue_scores(ksrc, c, p0, qc, pp):
                ps_s = s_ps.tile([P, 1024], f32, tag="s")
                for j in range(2):
                    kt = pp * 2 + j
                    nc.tensor.matmul(
                        ps_s[:, ts(j, 512)],
                        ksrc[p0:p0 + 64, ts(kt, P)],
                        qfm[c][p0:p0 + 64, ts(qc, 512)],
                        start=True, stop=True)
                return ps_s

            # Deferred softmax-normalize of the previous (h, qc) group: 1/den
            # on DVE, broadcast to 64 partitions via a K=1 bf16 matmul.  (DVE
            # ops may read at most ONE PSUM operand, so y is bounced through
            # SBUF.)  Called from inside the NEXT group's pair loop so the
            # broadcast matmul does not stall the in-order PE queue.
            pending = []

            def flush_normalize():
                while pending:
                    ps_y, c, p0, qc = pending.pop()
                    draw = npool.tile([65, 512], f32, tag="draw")
                    nc.vector.tensor_copy(draw[:], ps_y[:])
                    # reciprocal_approx_* misbehaves off partition 0, so hop
                    # the denominator row down first (cross-partition copy)
                    den0 = npool.tile([1, 512], f32, tag="den0")
                    nc.vector.tensor_copy(den0[:], draw[64:65, :])
                    rcp = npool.tile([1, 512], f32, tag="rcp")
                    nc.vector.reciprocal_approx_fast(rcp[:], den0[:])
                    den_bf = npool.tile([1, 512], bf16, tag="denb")
                    nc.vector.tensor_copy(den_bf[:], rcp[:])
                    ps_r = r_ps.tile([64, 512], f32, tag="r")
                    nc.tensor.matmul(ps_r[:], ones_bf[:], den_bf[:],
                                     start=True, stop=True)
                    nc.vector.tensor_tensor(yfm[c][p0:p0 + 64, ts(qc, 512)],
                                            draw[0:64, :], ps_r[:], OP.mult)

            # Out-projection, fused into the attention stream.  A group is
            # one (token tile, 512-col chunk): 4 accumulating matmuls + a DVE
            # PSUM->SBUF bounce + the output DMA.  Groups for q-chunk qc
            # become ready once all 8 heads have normalized qc; they are
            # interleaved one per pair-iteration of the NEXT qc as PE filler
            # (keeps the HAM clock gate warm through exp stalls and absorbs
            # what used to be a serial phase 3).
            owork = []

            def emit_outproj_group():
                t, oc = owork.pop(0)
                ps_o = o_ps.tile([P, 512], f32, tag="o")
                for yc in range(4):
                    nc.tensor.matmul(ps_o[:], yfm[yc][:, ts(t, P)],
                                     wo_sb[:, yc, ts(oc, 512)],
                                     start=(yc == 0), stop=(yc == 3))
                ob = opool.tile([P, 512], bf16, tag="ob")
                nc.vector.tensor_copy(ob[:], ps_o[:])
                nc.sync.dma_start(out[ts(t, P), ts(oc, 512)], ob[:])

            # flat (group, pair) schedule: scores for pair s+1 are issued
            # between exp(s) and PV(s) — across group boundaries too — so the
            # PE always has runway while ScalarE computes the current exp
            groups = []
            for qc in range(QC):
                # kswap is ready only a little after phase 1; run the heads
                # that read kfm directly first on the opening q-chunk
                horder = (0, 2, 5, 7, 1, 3, 4, 6) if qc == 0 else range(8)
                for h in horder:
                    kv, c, p0 = h // 4, h // 2, 64 * (h % 2)
                    nat = (kv == 0) == (p0 == 0)
                    nkt = 4 * qc + 4 if causal else 4 * QC
                    groups.append((qc, kv, c, p0,
                                   kfm if nat else kswap, nkt))
            sched = [(gi, pp) for gi, g in enumerate(groups)
                     for pp in range(g[5] // 2)]

            def issue_scores2(gi, pp):
                qc, kv, c, p0, ksrc, nkt = groups[gi]
                return issue_scores(ksrc, c, p0, qc, pp)

            pace = [0, 1]  # Bresenham accumulator / slots-per-emit denom
            ps_y_of = {}
            slot_s = {0: issue_scores2(*sched[0])}
            for si, (gi, pp) in enumerate(sched):
                qc, kv, c, p0, ksrc, nkt = groups[gi]
                npairs = nkt // 2
                ps_s = slot_s.pop(si)
                eg = epool.tile([P, 1024], bf16, tag="eg")
                nc.scalar.activation(eg[:], ps_s[:], AF.Exp)
                if causal and pp >= npairs - 2:
                    ppl = pp - (npairs - 2)  # 0 or 1 within diag quad
                    # keep where ktok <= q  <=>  q - i - 128j - 256ppl >= 0
                    nc.gpsimd.affine_select(
                        eg[:].rearrange("p (j q) -> p j q", q=512),
                        eg[:].rearrange("p (j q) -> p j q", q=512),
                        pattern=[[-128, 2], [1, 512]],
                        compare_op=OP.is_ge,
                        fill=0.0,
                        base=-256 * ppl,
                        channel_multiplier=-1)
                if si + 1 < len(sched):
                    slot_s[si + 1] = issue_scores2(*sched[si + 1])
                if pp == 1:
                    flush_normalize()
                elif owork and not pending:
                    # out-proj filler; valid only once the owning q-chunk's
                    # last normalize has flushed (pending empty)
                    pace[0] += 16
                    if pace[0] >= pace[1]:
                        pace[0] -= pace[1]
                        emit_outproj_group()
                if gi not in ps_y_of:
                    ps_y_of[gi] = y_ps.tile([65, 512], f32, tag="y",
                                            name=f"psy{gi}")
                ps_y = ps_y_of[gi]
                for j in range(2):
                    kt = pp * 2 + j
                    nc.tensor.matmul(
                        ps_y[:], vsb[kt][:, 65 * kv:65 * kv + 65],
                        eg[:, ts(j, 512)],
                        start=(kt == 0), stop=(kt == nkt - 1))
                if pp == npairs - 1:
                    pending.append((ps_y_of.pop(gi), c, p0, qc))
                    if gi + 1 == len(groups) or groups[gi + 1][0] != qc:
                        # qc complete: queue its out-projection groups and
                        # retune pacing for the next q-chunk's slot count
                        owork.extend((4 * qc + t4, oc)
                                     for t4 in range(4)
                                     for oc in range(D_MODEL // 512))
                        pace = [0, max(1, 8 * (2 * (qc + 1) + 1) - 1)]
            flush_normalize()
        # drain the last q-chunk's out-projection with deeper PSUM buffering
        # (the attention PSUM pools are closed, freeing their banks)
        with ExitStack() as p3:
            dpool = p3.enter_context(tc.tile_pool(name="drain", bufs=4))
            d_ps = p3.enter_context(
                tc.tile_pool(name="dpsum", bufs=4, space="PSUM"))
            while owork:
                t, oc = owork.pop(0)
                ps_o = d_ps.tile([P, 512], f32, tag="o")
                for yc in range(4):
                    nc.tensor.matmul(ps_o[:], yfm[yc][:, ts(t, P)],
                                     wo_sb[:, yc, ts(oc, 512)],
                                     start=(yc == 0), stop=(yc == 3))
                ob = dpool.tile([P, 512], bf16, tag="ob")
                nc.vector.tensor_copy(ob[:], ps_o[:])
                nc.sync.dma_start(out[ts(t, P), ts(oc, 512)], ob[:])

    nc.compile()
    return nc


def _rope_tables(pos, norm_w, scale):
    """Build [P, NT, 4, 32] tables A,B,C,D for out1 = t1*A - t2*B,
    out2 = t1*C + t2*D (NeoX rope with folded norm weight + score scale)."""
    n_tok = pos.shape[0]
    f = np.arange(0, D_HEAD, 2, dtype=np.float64) / D_HEAD
    inv_freq = 1.0 / (ROPE_BASE ** f)                       # [32]
    ang = pos.astype(np.float64)[:, None] * inv_freq[None, :]  # [n, 32]
    cos, sin = np.cos(ang), np.sin(ang)
    w1 = norm_w[:32].astype(np.float64)
    w2 = norm_w[32:].astype(np.float64)
    A = cos * w1 * scale
    Bt = sin * w2 * scale
    C = sin * w1 * scale
    D = cos * w2 * scale
    tab = np.stack([A, Bt, C, D], axis=1).astype(np.float32)  # [n, 4, 32]
    return np.ascontiguousarray(
        tab.reshape(n_tok // P, P, 4, 32).transpose(1, 0, 2, 3))


def make_in_maps(x, pos, qkv_w, out_w, q_norm_w, k_norm_w, n_tok=N):
    import ml_dtypes
    bf = ml_dtypes.bfloat16

    scale = D_HEAD ** -0.5
    tabq = _rope_tables(pos, q_norm_w, scale)
    tabk = _rope_tables(pos, k_norm_w, 1.0)
    wq_all = qkv_w[0:H_Q * D_HEAD].reshape(H_Q, D_HEAD, D_MODEL)
    wk_all = qkv_w[H_Q * D_HEAD:(H_Q + H_KV) * D_HEAD].reshape(
        H_KV, D_HEAD, D_MODEL)
    wv_all = qkv_w[(H_Q + H_KV) * D_HEAD:].reshape(H_KV, D_HEAD, D_MODEL)
    wo_all = out_w.reshape(D_MODEL, H_Q, D_HEAD)

    in_maps = []
    for c in range(NCORES):
        b, hg = divmod(c, 4)
        heads = list(range(8 * hg, 8 * hg + 8))
        kvs = [2 * hg, 2 * hg + 1]
        wsel = np.concatenate([
            wq_all[heads].reshape(512, D_MODEL),
            wk_all[kvs].reshape(128, D_MODEL),
            wv_all[kvs].reshape(128, D_MODEL)], axis=0)    # [768, D]
        in_maps.append({
            "xT": np.ascontiguousarray(x[b].T).astype(bf),
            "wqkv": np.ascontiguousarray(wsel.T).astype(bf),
            "wo": np.ascontiguousarray(
                wo_all[:, heads].reshape(D_MODEL, 512).T).astype(bf),
            "tabq": tabq,
            "tabk": tabk,
        })
    return in_maps


def _reference_host(x, mask, pos, qkv_w, out_w, q_norm_w, k_norm_w):
    """Pure-numpy fallback, used only if the mask is not causal."""
    xx = x.astype(np.float64)
    qkv = xx @ qkv_w.T.astype(np.float64)
    Bsz, Nl, _ = x.shape
    qkv = qkv.reshape(Bsz, Nl, H_Q + 2 * H_KV, D_HEAD).transpose(0, 2, 1, 3)
    q, k, v = (qkv[:, :H_Q], qkv[:, H_Q:H_Q + H_KV], qkv[:, H_Q + H_KV:])

    def rms(t, w):
        var = np.mean(t * t, axis=-1, keepdims=True)
        return t / np.sqrt(var + EPS) * w

    def rope(t):
        f = np.arange(0, D_HEAD, 2) / D_HEAD
        inv = 1.0 / (ROPE_BASE ** f)
        ang = pos.astype(np.float64)[:, None] * inv[None, :]
        cs, sn = np.cos(ang), np.sin(ang)
        t1, t2 = t[..., :32], t[..., 32:]
        return np.concatenate([t1 * cs - t2 * sn, t1 * sn + t2 * cs], axis=-1)

    q, k = rope(rms(q, q_norm_w)), rope(rms(k, k_norm_w))
    qg = q.reshape(Bsz, H_KV, 4, Nl, D_HEAD)
    sc = np.einsum("bhgnd,bhmd->bhgnm", qg, k) * (D_HEAD ** -0.5)
    sc = np.where(mask[None, None, None], -np.inf, sc)
    sc -= sc.max(axis=-1, keepdims=True)
    p = np.exp(sc)
    p /= p.sum(axis=-1, keepdims=True)
    y = np.einsum("bhgnm,bhmd->bhgnd", p, v)
    y = y.reshape(Bsz, H_Q, Nl, D_HEAD).transpose(0, 2, 1, 3).reshape(
        Bsz, Nl, D_MODEL)
    return (y @ out_w.T.astype(np.float64)).astype(np.float32)


_NC_CACHE = {}


def run_on_device(in_maps, n_tok=N, trace=False, trace_kwargs=None):
    import sys
    for p in ("/opt/trn_rl_repo",):
        if p not in sys.path:
            sys.path.insert(0, p)
    from concourse.bass_utils import run_bass_kernel_spmd

    key = n_tok
    if key not in _NC_CACHE:
        _NC_CACHE[key] = build_nc(n_tok)
    nc = _NC_CACHE[key]
    return run_bass_kernel_spmd(
        nc, in_maps, list(range(len(in_maps))), trace=trace,
        **(trace_kwargs or {}))


def kernel(x, mask, pos, qkv_w, out_w, q_norm_w, k_norm_w):
    x = np.asarray(x, dtype=np.float32)
    mask = np.asarray(mask)
    pos = np.asarray(pos)
    causal = bool(
        np.array_equal(mask,
                       np.triu(np.ones((N, N), dtype=bool), k=1)))
    if not causal:
        return _reference_host(x, mask, pos, np.asarray(qkv_w),
                               np.asarray(out_w), np.asarray(q_norm_w),
                               np.asarray(k_norm_w))
    in_maps = make_in_maps(x, pos, np.asarray(qkv_w, dtype=np.float32),
                           np.asarray(out_w, dtype=np.float32),
                           np.asarray(q_norm_w, dtype=np.float32),
                           np.asarray(k_norm_w, dtype=np.float32))
    res = run_on_device(in_maps)
    outs = [np.asarray(r["out"], dtype=np.float32) for r in res.results]
    full = np.empty((B, N, D_MODEL), dtype=np.float32)
    for b in range(B):
        full[b] = outs[4 * b] + outs[4 * b + 1] + outs[4 * b + 2] + outs[4 * b + 3]
    return full


# revision 40
# speedup vs baseline: 1.0596x; 1.0580x over previous
"""GQA attention block (B=2, N=2048, D=2048, Hq=32, Hkv=8, d=64) on 8 TRN2 NeuronCores.

Sharding: core c = b*4 + hg  (data-parallel over batch b in {0,1}; tensor-parallel
over 4 head-groups hg, each owning 8 q-heads / 2 kv-heads).  Each core computes a
row-parallel partial of the output projection for its batch; the host sums the 4
partials per batch.

Per-core device pipeline (matmuls in bf16, PSUM accumulation fp32):
  1) token-major fused QKV projection:  psum[tok128, 768] = x_chunk.T @ Wqkv.T
  2) RMSNorm (free-dim segment reduce) + NeoX RoPE via 4 host-folded tables
     (q tables also absorb the 1/sqrt(d) score scale and q_norm_w; k tables absorb
     k_norm_w) in fp32
  3) PE transpose of rotated q,k to feature-major, cast to bf16 on the copy out
  4) per (head, 512-token q-chunk): scoresT[k,q] matmuls (K=64) in bf16, exp on
     ScalarE (no max-subtraction: scores are O(5) so exp is safe) -> bf16, causal
     mask via GPSIMD affine_select, PV matmul with an appended ones-column on V
     producing y and the softmax denominator in one accumulation; normalize with
     DVE reciprocal + a K=1 bf16 broadcast matmul
  5) out-projection partial (bf16 matmul), interleaved into the attention
     stream as PE filler, token-major, DMA'd out in bf16 (host sums in fp32)
"""

import numpy as np

D_MODEL = 2048
H_Q, H_KV, D_HEAD = 32, 8, 64
B = 2
N = 2048
ROPE_BASE = 10000.0
EPS = 1e-6
NCORES = 8
P = 128


def _modules():
    import sys

    for p in ("/opt/trn_rl_repo",):
        if p not in sys.path:
            sys.path.insert(0, p)
    import concourse.bass as bass
    import concourse.tile as tile
    from concourse import bacc, mybir
    from concourse.masks import make_identity

    return bass, tile, bacc, mybir, make_identity


def build_nc(n_tok=N, causal=True, dbg=False):
    """Build the single-core SPMD Bass program (identical on all 8 cores)."""
    from contextlib import ExitStack

    bass, tile, bacc, mybir, make_identity = _modules()
    f32 = mybir.dt.float32
    bf16 = mybir.dt.bfloat16
    ts = bass.ts
    AF = mybir.ActivationFunctionType
    OP = mybir.AluOpType

    NT = n_tok // P           # token tiles
    DC = D_MODEL // P         # contraction chunks for qkv proj
    QC = n_tok // 512         # query chunks of 512
    NG = n_tok // 512         # x-load groups (512 tokens each)
    assert QC >= 1 and n_tok % 512 == 0

    nc = bacc.Bacc("TRN2", target_bir_lowering=False, debug=False,
                   num_devices=NCORES)

    xT = nc.dram_tensor("xT", [D_MODEL, n_tok], bf16, kind="ExternalInput").ap()
    wqkv = nc.dram_tensor("wqkv", [D_MODEL, 768], bf16, kind="ExternalInput").ap()
    wo = nc.dram_tensor("wo", [512, D_MODEL], bf16, kind="ExternalInput").ap()
    tabq = nc.dram_tensor("tabq", [P, NT, 4, 32], f32, kind="ExternalInput").ap()
    tabk = nc.dram_tensor("tabk", [P, NT, 4, 32], f32, kind="ExternalInput").ap()
    out = nc.dram_tensor("out", [n_tok, D_MODEL], bf16,
                         kind="ExternalOutput").ap()

    with ExitStack() as ctx:
        tc = ctx.enter_context(tile.TileContext(nc))

        cpool = ctx.enter_context(tc.tile_pool(name="const", bufs=1))
        # persistent activations (bf16 matmul operands)
        qfm = [cpool.tile([P, n_tok], bf16, name=f"qfm{c}") for c in range(4)]
        kfm = cpool.tile([P, n_tok], bf16, name="kfm")     # [kv0 | kv1] on partitions
        kswap = cpool.tile([P, n_tok], bf16, name="kswap")  # [kv1 | kv0]
        yfm = [cpool.tile([P, n_tok], bf16, name=f"yfm{c}") for c in range(4)]
        vsb = [cpool.tile([P, 130], bf16, name=f"vsb{t}") for t in range(NT)]
        ident = cpool.tile([P, P], f32, name="ident")
        make_identity(nc, ident[:])
        ones_bf = cpool.tile([1, 64], bf16, name="ones_bf")
        nc.gpsimd.memset(ones_bf[:], 1.0)
        eps_t = cpool.tile([P, 1], f32, name="eps_t")
        nc.gpsimd.memset(eps_t[:], EPS)
        for t in range(NT):
            nc.gpsimd.memset(vsb[t][:, 64:65], 1.0)
            nc.gpsimd.memset(vsb[t][:, 129:130], 1.0)

        # ---------------- phase 1: qkv + norm + rope + transpose ----------
        with ExitStack() as p1:
            wpool = p1.enter_context(tc.tile_pool(name="wqkv", bufs=1))
            tpool = p1.enter_context(tc.tile_pool(name="tabs", bufs=1))
            xpool = p1.enter_context(tc.tile_pool(name="xg", bufs=9))
            wkk = p1.enter_context(tc.tile_pool(name="qkvwork", bufs=2))
            qkv_ps = p1.enter_context(
                tc.tile_pool(name="qkvpsum", bufs=2, space="PSUM"))
            tp_ps = p1.enter_context(
                tc.tile_pool(name="tppsum", bufs=2, space="PSUM"))

            # DMA order matters: the first QKV matmul needs x group 0 and the
            # first weight chunk, so issue those ahead of the big table loads
            # (all DMAs drain through one issue queue).  Weights are one tile
            # per contraction chunk so the accumulation can start as soon as
            # chunk 0 lands (tile-granular dependency tracking).
            wq_sb = [wpool.tile([P, 4, 768], bf16, name=f"wq{i}")
                     for i in range(4)]
            wq_src = wqkv.rearrange("(o p) r -> p o r", p=P)
            x_src = xT.rearrange("(dc p) t -> p dc t", p=P)
            xg0 = xpool.tile([P, DC, 256], bf16, tag="xg", name="xg0")
            nc.sync.dma_start(xg0[:], x_src[:, :, ts(0, 256)])
            for i in range(4):
                nc.sync.dma_start(wq_sb[i][:], wq_src[:, ts(i, 4), :])
            # rope tables as 4 chunk-tiles each, so tile 0's rope only gates
            # on the first 512 KiB; later chunks load behind x group 1
            TC4 = NT // 2
            tq = [tpool.tile([P, TC4, 4, 32], f32, name=f"tq{i}")
                  for i in range(2)]
            tk = [tpool.tile([P, TC4, 4, 32], f32, name=f"tk{i}")
                  for i in range(2)]
            nc.sync.dma_start(tq[0][:], tabq[:, ts(0, TC4)])
            nc.sync.dma_start(tk[0][:], tabk[:, ts(0, TC4)])

            for g in range(2 * NG):
                if g == 0:
                    xg = xg0
                else:
                    xg = xpool.tile([P, DC, 256], bf16, tag="xg",
                                    name=f"xg{g}")
                    nc.sync.dma_start(xg[:], x_src[:, :, ts(g, 256)])
                if g == 1:
                    nc.sync.dma_start(tq[1][:], tabq[:, ts(1, TC4)])
                    nc.sync.dma_start(tk[1][:], tabk[:, ts(1, TC4)])
                for lt in range(2):
                    tt = g * 2 + lt
                    ps = qkv_ps.tile([P, 768], f32, tag="qkv")
                    for dc in range(DC):
                        lhsT = xg[:, dc, ts(lt, P)]
                        wqc = wq_sb[dc // 4][:, dc % 4]
                        nc.tensor.matmul(ps[:, 0:512], lhsT, wqc[:, 0:512],
                                         start=(dc == 0), stop=(dc == DC - 1))
                        nc.tensor.matmul(ps[:, 512:768], lhsT, wqc[:, 512:768],
                                         start=(dc == 0), stop=(dc == DC - 1))
                    # --- rmsnorm ---
                    sq = wkk.tile([P, 512], f32, tag="sq")
                    nc.scalar.activation(sq[:], ps[:, 0:512], AF.Square)
                    sqk = wkk.tile([P, 128], f32, tag="sqk")
                    nc.scalar.activation(sqk[:], ps[:, 512:640], AF.Square)
                    ssq = wkk.tile([P, 10], f32, tag="ssq")
                    nc.vector.reduce_sum(
                        ssq[:, 0:8], sq[:].rearrange("p (h d) -> p h d", d=64),
                        axis=mybir.AxisListType.X)
                    nc.vector.reduce_sum(
                        ssq[:, 8:10], sqk[:].rearrange("p (h d) -> p h d", d=64),
                        axis=mybir.AxisListType.X)
                    sd = wkk.tile([P, 10], f32, tag="sd")
                    nc.scalar.activation(sd[:], ssq[:], AF.Sqrt,
                                         bias=eps_t[:], scale=1.0 / 64)
                    rs = wkk.tile([P, 10], f32, tag="rs")
                    nc.vector.reciprocal_approx_fast(rs[:], sd[:])
                    qn = wkk.tile([P, 512], f32, tag="sq")
                    nc.vector.tensor_tensor(
                        qn[:].rearrange("p (h d) -> p h d", d=64),
                        ps[:, 0:512].rearrange("p (h d) -> p h d", d=64),
                        rs[:, 0:8, None].to_broadcast([P, 8, 64]), OP.mult)
                    kn = wkk.tile([P, 128], f32, tag="kn")
                    nc.vector.tensor_tensor(
                        kn[:].rearrange("p (h d) -> p h d", d=64),
                        ps[:, 512:640].rearrange("p (h d) -> p h d", d=64),
                        rs[:, 8:10, None].to_broadcast([P, 2, 64]), OP.mult)
                    # --- v copy (bf16 cast, with ones cols at 64 and 129) ---
                    nc.scalar.activation(vsb[tt][:, 0:64], ps[:, 640:704], AF.Copy)
                    nc.scalar.activation(vsb[tt][:, 65:129], ps[:, 704:768], AF.Copy)
                    # --- rope (fp32) ---
                    qr = wkk.tile([P, 512], f32, tag="qr")
                    kr = wkk.tile([P, 128], f32, tag="kr")
                    for (src, dst, tabs, nh) in ((qn, qr, tq, 8),
                                                 (kn, kr, tk, 2)):
                        sv = src[:].rearrange("p (h d) -> p h d", d=64)
                        dv = dst[:].rearrange("p (h d) -> p h d", d=64)
                        t1, t2 = sv[:, :, 0:32], sv[:, :, 32:64]
                        tab, tl = tabs[tt // TC4], tt % TC4
                        A = tab[:, tl, 0:1, :].to_broadcast([P, nh, 32])
                        Bt = tab[:, tl, 1:2, :].to_broadcast([P, nh, 32])
                        C = tab[:, tl, 2:3, :].to_broadcast([P, nh, 32])
                        D = tab[:, tl, 3:4, :].to_broadcast([P, nh, 32])
                        u1 = wkk.tile([P, nh, 32], f32, tag=f"u1_{nh}")
                        u2 = wkk.tile([P, nh, 32], f32, tag=f"u2_{nh}")
                        nc.vector.tensor_tensor(u1[:], t1, A, OP.mult)
                        nc.vector.tensor_tensor(u2[:], t2, Bt, OP.mult)
                        nc.vector.tensor_tensor(dv[:, :, 0:32], u1[:], u2[:],
                                                OP.subtract)
                        u3 = wkk.tile([P, nh, 32], f32, tag=f"u1_{nh}")
                        u4 = wkk.tile([P, nh, 32], f32, tag=f"u2_{nh}")
                        nc.vector.tensor_tensor(u3[:], t1, C, OP.mult)
                        nc.vector.tensor_tensor(u4[:], t2, D, OP.mult)
                        nc.vector.tensor_tensor(dv[:, :, 32:64], u3[:], u4[:],
                                                OP.add)
                    # --- transpose to feature-major (fp32 PE transpose,
                    #     bf16 cast on the PSUM->SBUF copy) ---
                    for rc in range(4):
                        pt = tp_ps.tile([P, P], f32, tag="tp")
                        nc.tensor.transpose(pt[:], qr[:, ts(rc, P)], ident[:])
                        nc.vector.tensor_copy(qfm[rc][:, ts(tt, P)], pt[:])
                    pt = tp_ps.tile([P, P], f32, tag="tp")
                    nc.tensor.transpose(pt[:], kr[:], ident[:])
                    nc.vector.tensor_copy(kfm[:, ts(tt, P)], pt[:])
            # kswap = partition halves of kfm exchanged (SBUF->SBUF DMA)
            nc.sync.dma_start(kswap[64:128, :], kfm[0:64, :])
            nc.sync.dma_start(kswap[0:64, :], kfm[64:128, :])

        # ---------------- phase 2: attention --------------------------
        wopool = ctx.enter_context(tc.tile_pool(name="wo", bufs=1))
        # allocated here (after phase-1 pools closed) so its SBUF reservation
        # does not overlap the phase-1 peak
        wo_sb = wopool.tile([P, 4, D_MODEL], bf16, name="wo_sb")
        nc.sync.dma_start(wo_sb[:], wo.rearrange("(o p) d -> p o d", p=P))
        with ExitStack() as p2:
            epool = p2.enter_context(tc.tile_pool(name="exp", bufs=4))
            npool = p2.enter_context(tc.tile_pool(name="nrm", bufs=4))
            opool = p2.enter_context(tc.tile_pool(name="osb", bufs=3))
            s_ps = p2.enter_context(
                tc.tile_pool(name="spsum", bufs=2, space="PSUM"))
            y_ps = p2.enter_context(
                tc.tile_pool(name="ypsum", bufs=2, space="PSUM"))
            r_ps = p2.enter_context(
                tc.tile_pool(name="rpsum", bufs=1, space="PSUM"))
            o_ps = p2.enter_context(
                tc.tile_pool(name="opsum", bufs=1, space="PSUM"))

            def issue_scores(ksrc, c, p0, qc, pp):
                ps_s = s_ps.tile([P, 1024], f32, tag="s")
                for j in range(2):
                    kt = pp * 2 + j
                    nc.tensor.matmul(
                        ps_s[:, ts(j, 512)],
                        ksrc[p0:p0 + 64, ts(kt, P)],
                        qfm[c][p0:p0 + 64, ts(qc, 512)],
                        start=True, stop=True)
                return ps_s

            # Deferred softmax-normalize of the previous (h, qc) group: 1/den
            # on DVE, broadcast to 64 partitions via a K=1 bf16 matmul.  (DVE
            # ops may read at most ONE PSUM operand, so y is bounced through
            # SBUF.)  Called from inside the NEXT group's pair loop so the
            # broadcast matmul does not stall the in-order PE queue.
            pending = []

            def flush_normalize():
                while pending:
                    ps_y, c, p0, qc = pending.pop()
                    draw = npool.tile([65, 512], f32, tag="draw")
                    nc.vector.tensor_copy(draw[:], ps_y[:])
                    # reciprocal_approx_* misbehaves off partition 0, so hop
                    # the denominator row down first (cross-partition copy)
                    den0 = npool.tile([1, 512], f32, tag="den0")
                    nc.vector.tensor_copy(den0[:], draw[64:65, :])
                    rcp = npool.tile([1, 512], f32, tag="rcp")
                    nc.vector.reciprocal_approx_fast(rcp[:], den0[:])
                    den_bf = npool.tile([1, 512], bf16, tag="denb")
                    nc.vector.tensor_copy(den_bf[:], rcp[:])
                    ps_r = r_ps.tile([64, 512], f32, tag="r")
                    nc.tensor.matmul(ps_r[:], ones_bf[:], den_bf[:],
                                     start=True, stop=True)
                    nc.vector.tensor_tensor(yfm[c][p0:p0 + 64, ts(qc, 512)],
                                            draw[0:64, :], ps_r[:], OP.mult)

            # Out-projection, fused into the attention stream.  A group is
            # one (token tile, 512-col chunk): 4 accumulating matmuls + a DVE
            # PSUM->SBUF bounce + the output DMA.  Groups for q-chunk qc
            # become ready once all 8 heads have normalized qc; they are
            # interleaved one per pair-iteration of the NEXT qc as PE filler
            # (keeps the HAM clock gate warm through exp stalls and absorbs
            # what used to be a serial phase 3).
            owork = []

            def emit_outproj_group():
                t, oc = owork.pop(0)
                ps_o = o_ps.tile([P, 512], f32, tag="o")
                for yc in range(4):
                    nc.tensor.matmul(ps_o[:], yfm[yc][:, ts(t, P)],
                                     wo_sb[:, yc, ts(oc, 512)],
                                     start=(yc == 0), stop=(yc == 3))
                ob = opool.tile([P, 512], bf16, tag="ob")
                nc.vector.tensor_copy(ob[:], ps_o[:])
                nc.sync.dma_start(out[ts(t, P), ts(oc, 512)], ob[:])

            # flat (group, pair) schedule: scores for pair s+1 are issued
            # between exp(s) and PV(s) — across group boundaries too — so the
            # PE always has runway while ScalarE computes the current exp
            groups = []
            for qc in range(QC):
                # kswap is ready only a little after phase 1; run the heads
                # that read kfm directly first on the opening q-chunk
                horder = (0, 2, 5, 7, 1, 3, 4, 6) if qc == 0 else range(8)
                for h in horder:
                    kv, c, p0 = h // 4, h // 2, 64 * (h % 2)
                    nat = (kv == 0) == (p0 == 0)
                    nkt = 4 * qc + 4 if causal else 4 * QC
                    groups.append((qc, kv, c, p0,
                                   kfm if nat else kswap, nkt))
            sched = [(gi, pp) for gi, g in enumerate(groups)
                     for pp in range(g[5] // 2)]

            def issue_scores2(gi, pp):
                qc, kv, c, p0, ksrc, nkt = groups[gi]
                return issue_scores(ksrc, c, p0, qc, pp)

            pace = [0, 1]  # Bresenham accumulator / slots-per-emit denom
            ps_y_of = {}
            slot_s = {0: issue_scores2(*sched[0])}
            for si, (gi, pp) in enumerate(sched):
                qc, kv, c, p0, ksrc, nkt = groups[gi]
                npairs = nkt // 2
                ps_s = slot_s.pop(si)
                eg = epool.tile([P, 1024], bf16, tag="eg")
                nc.scalar.activation(eg[:], ps_s[:], AF.Exp)
                if causal and pp >= npairs - 2:
                    ppl = pp - (npairs - 2)  # 0 or 1 within diag quad
                    # keep where ktok <= q  <=>  q - i - 128j - 256ppl >= 0
                    nc.gpsimd.affine_select(
                        eg[:].rearrange("p (j q) -> p j q", q=512),
                        eg[:].rearrange("p (j q) -> p j q", q=512),
                        pattern=[[-128, 2], [1, 512]],
                        compare_op=OP.is_ge,
                        fill=0.0,
                        base=-256 * ppl,
                        channel_multiplier=-1)
                if si + 1 < len(sched):
                    slot_s[si + 1] = issue_scores2(*sched[si + 1])
                if pp == (1 if npairs <= 2 else 2):
                    flush_normalize()
                elif owork and not pending:
                    # out-proj filler; valid only once the owning q-chunk's
                    # last normalize has flushed (pending empty)
                    pace[0] += 16
                    if pace[0] >= pace[1]:
                        pace[0] -= pace[1]
                        emit_outproj_group()
                if gi not in ps_y_of:
                    ps_y_of[gi] = y_ps.tile([65, 512], f32, tag="y",
                                            name=f"psy{gi}")
                ps_y = ps_y_of[gi]
                for j in range(2):
                    kt = pp * 2 + j
                    nc.tensor.matmul(
                        ps_y[:], vsb[kt][:, 65 * kv:65 * kv + 65],
                        eg[:, ts(j, 512)],
                        start=(kt == 0), stop=(kt == nkt - 1))
                if pp == npairs - 1:
                    pending.append((ps_y_of.pop(gi), c, p0, qc))
                    if gi + 1 == len(groups) or groups[gi + 1][0] != qc:
                        # qc complete: queue its out-projection groups and
                        # retune pacing for the next q-chunk's slot count
                        owork.extend((4 * qc + t4, oc)
                                     for t4 in range(4)
                                     for oc in range(D_MODEL // 512))
                        pace = [0, max(1, 8 * (2 * (qc + 1) + 1) - 1)]
            flush_normalize()
        # drain the last q-chunk's out-projection with deeper PSUM buffering
        # (the attention PSUM pools are closed, freeing their banks)
        with ExitStack() as p3:
            dpool = p3.enter_context(tc.tile_pool(name="drain", bufs=4))
            d_ps = p3.enter_context(
                tc.tile_pool(name="dpsum", bufs=4, space="PSUM"))
            while owork:
                t, oc = owork.pop(0)
                ps_o = d_ps.tile([P, 512], f32, tag="o")
                for yc in range(4):
                    nc.tensor.matmul(ps_o[:], yfm[yc][:, ts(t, P)],
                                     wo_sb[:, yc, ts(oc, 512)],
                                     start=(yc == 0), stop=(yc == 3))
                ob = dpool.tile([P, 512], bf16, tag="ob")
                nc.vector.tensor_copy(ob[:], ps_o[:])
                nc.sync.dma_start(out[ts(t, P), ts(oc, 512)], ob[:])

    nc.compile()
    return nc


def _rope_tables(pos, norm_w, scale):
    """Build [P, NT, 4, 32] tables A,B,C,D for out1 = t1*A - t2*B,
    out2 = t1*C + t2*D (NeoX rope with folded norm weight + score scale)."""
    n_tok = pos.shape[0]
    f = np.arange(0, D_HEAD, 2, dtype=np.float64) / D_HEAD
    inv_freq = 1.0 / (ROPE_BASE ** f)                       # [32]
    ang = pos.astype(np.float64)[:, None] * inv_freq[None, :]  # [n, 32]
    cos, sin = np.cos(ang), np.sin(ang)
    w1 = norm_w[:32].astype(np.float64)
    w2 = norm_w[32:].astype(np.float64)
    A = cos * w1 * scale
    Bt = sin * w2 * scale
    C = sin * w1 * scale
    D = cos * w2 * scale
    tab = np.stack([A, Bt, C, D], axis=1).astype(np.float32)  # [n, 4, 32]
    return np.ascontiguousarray(
        tab.reshape(n_tok // P, P, 4, 32).transpose(1, 0, 2, 3))


def make_in_maps(x, pos, qkv_w, out_w, q_norm_w, k_norm_w, n_tok=N):
    import ml_dtypes
    bf = ml_dtypes.bfloat16

    scale = D_HEAD ** -0.5
    tabq = _rope_tables(pos, q_norm_w, scale)
    tabk = _rope_tables(pos, k_norm_w, 1.0)
    wq_all = qkv_w[0:H_Q * D_HEAD].reshape(H_Q, D_HEAD, D_MODEL)
    wk_all = qkv_w[H_Q * D_HEAD:(H_Q + H_KV) * D_HEAD].reshape(
        H_KV, D_HEAD, D_MODEL)
    wv_all = qkv_w[(H_Q + H_KV) * D_HEAD:].reshape(H_KV, D_HEAD, D_MODEL)
    wo_all = out_w.reshape(D_MODEL, H_Q, D_HEAD)

    in_maps = []
    for c in range(NCORES):
        b, hg = divmod(c, 4)
        heads = list(range(8 * hg, 8 * hg + 8))
        kvs = [2 * hg, 2 * hg + 1]
        wsel = np.concatenate([
            wq_all[heads].reshape(512, D_MODEL),
            wk_all[kvs].reshape(128, D_MODEL),
            wv_all[kvs].reshape(128, D_MODEL)], axis=0)    # [768, D]
        in_maps.append({
            "xT": np.ascontiguousarray(x[b].T).astype(bf),
            "wqkv": np.ascontiguousarray(wsel.T).astype(bf),
            "wo": np.ascontiguousarray(
                wo_all[:, heads].reshape(D_MODEL, 512).T).astype(bf),
            "tabq": tabq,
            "tabk": tabk,
        })
    return in_maps


def _reference_host(x, mask, pos, qkv_w, out_w, q_norm_w, k_norm_w):
    """Pure-numpy fallback, used only if the mask is not causal."""
    xx = x.astype(np.float64)
    qkv = xx @ qkv_w.T.astype(np.float64)
    Bsz, Nl, _ = x.shape
    qkv = qkv.reshape(Bsz, Nl, H_Q + 2 * H_KV, D_HEAD).transpose(0, 2, 1, 3)
    q, k, v = (qkv[:, :H_Q], qkv[:, H_Q:H_Q + H_KV], qkv[:, H_Q + H_KV:])

    def rms(t, w):
        var = np.mean(t * t, axis=-1, keepdims=True)
        return t / np.sqrt(var + EPS) * w

    def rope(t):
        f = np.arange(0, D_HEAD, 2) / D_HEAD
        inv = 1.0 / (ROPE_BASE ** f)
        ang = pos.astype(np.float64)[:, None] * inv[None, :]
        cs, sn = np.cos(ang), np.sin(ang)
        t1, t2 = t[..., :32], t[..., 32:]
        return np.concatenate([t1 * cs - t2 * sn, t1 * sn + t2 * cs], axis=-1)

    q, k = rope(rms(q, q_norm_w)), rope(rms(k, k_norm_w))
    qg = q.reshape(Bsz, H_KV, 4, Nl, D_HEAD)
    sc = np.einsum("bhgnd,bhmd->bhgnm", qg, k) * (D_HEAD ** -0.5)
    sc = np.where(mask[None, None, None], -np.inf, sc)
    sc -= sc.max(axis=-1, keepdims=True)
    p = np.exp(sc)
    p /= p.sum(axis=-1, keepdims=True)
    y = np.einsum("bhgnm,bhmd->bhgnd", p, v)
    y = y.reshape(Bsz, H_Q, Nl, D_HEAD).transpose(0, 2, 1, 3).reshape(
        Bsz, Nl, D_MODEL)
    return (y @ out_w.T.astype(np.float64)).astype(np.float32)


_NC_CACHE = {}


def run_on_device(in_maps, n_tok=N, trace=False, trace_kwargs=None):
    import sys
    for p in ("/opt/trn_rl_repo",):
        if p not in sys.path:
            sys.path.insert(0, p)
    from concourse.bass_utils import run_bass_kernel_spmd

    key = n_tok
    if key not in _NC_CACHE:
        _NC_CACHE[key] = build_nc(n_tok)
    nc = _NC_CACHE[key]
    return run_bass_kernel_spmd(
        nc, in_maps, list(range(len(in_maps))), trace=trace,
        **(trace_kwargs or {}))


def kernel(x, mask, pos, qkv_w, out_w, q_norm_w, k_norm_w):
    x = np.asarray(x, dtype=np.float32)
    mask = np.asarray(mask)
    pos = np.asarray(pos)
    causal = bool(
        np.array_equal(mask,
                       np.triu(np.ones((N, N), dtype=bool), k=1)))
    if not causal:
        return _reference_host(x, mask, pos, np.asarray(qkv_w),
                               np.asarray(out_w), np.asarray(q_norm_w),
                               np.asarray(k_norm_w))
    in_maps = make_in_maps(x, pos, np.asarray(qkv_w, dtype=np.float32),
                           np.asarray(out_w, dtype=np.float32),
                           np.asarray(q_norm_w, dtype=np.float32),
                           np.asarray(k_norm_w, dtype=np.float32))
    res = run_on_device(in_maps)
    outs = [np.asarray(r["out"], dtype=np.float32) for r in res.results]
    full = np.empty((B, N, D_MODEL), dtype=np.float32)
    for b in range(B):
        full[b] = outs[4 * b] + outs[4 * b + 1] + outs[4 * b + 2] + outs[4 * b + 3]
    return full
